# revision 61
# baseline (speedup 1.0000x reference)
# Trainium2 Bass kernel for GQA causal attention (B=2, S=2048, DIM=2048,
# NH=32, NKV=8, HD=64) sharded over 8 NeuronCores: 2-way data parallel over
# batch x 4-way tensor parallel over heads. Each core computes 8 query heads
# (2 KV heads) for one batch element plus a partial wo product; the partial
# sums are reduced on the host (cheap fp32 adds), so no device collective is
# needed.
#
# Structure: per chunk, K/V projection chains run first (so kt/vp are ready
# the moment attention starts), then Q chains + RoPE into per-chunk query
# tiles (no false cross-chunk deps). Attention uses merged [128,1024] score
# PSUM tiles (both heads of a GQA pair) with a single EXP per key tile and a
# software-pipelined t-loop (scores for t+1 emitted before AV of t so a
# dep-waiting AV never blocks the in-order PE queue). Softmax normalization
# is batched per chunk: one [8,512] reciprocal over all denominator rows
# (gathered by SBUF->SBUF DMAs; compute engines cannot write partitions at
# base 1..7), broadcast via K=8 selector matmuls. The whole data path is
# bf16 (fp32 PSUM accumulation), which enables FWL weight loads and 2x DVE
# modes; wo stays resident in SBUF. Measured ~404us on-device (NTFF).
#
# Self-contained: hardcodes all shapes; only imports the concourse runtime
# available in the environment.
import os
import ml_dtypes
import numpy as np

B, S, DIM = 2, 2048, 2048
NH, NKV, HD = 32, 8, 64
THETA = 10000.0
TPG = 4               # tensor-parallel head-group shards
H_CORE = NH // TPG    # 8 query heads per core
KV_CORE = NKV // TPG  # 2 kv heads per core
SCH = 512             # sequence chunk (matmul moving free dim)
NSCH = S // SCH       # 4
DT = DIM // 128       # 16 contraction tiles for projections
ST = S // 128         # 16 key tiles
N_CORES = 8

# within-head dim permutation: [e0(16) o0(16) e1(16) o1(16)] so that the RoPE
# partner lives 16 partitions away inside each 32-partition quadrant
# (stream_shuffle shuffles within 32-partition quadrants only).
PERM64 = np.array([2 * i for i in range(16)] + [2 * i + 1 for i in range(16)]
                  + [32 + 2 * i for i in range(16)]
                  + [33 + 2 * i for i in range(16)])
HEAD_ORDER_LOCAL = [0, 4, 1, 5, 2, 6, 3, 7]  # (p, p+4) share a 128-row tile
SHUF_MASK = [i ^ 16 for i in range(32)]

_CACHE: dict = {}
LAST_RUN_INFO: dict = {}


def _host_constants():
    freqs = 1.0 / (THETA ** (np.arange(0, HD, 2, dtype=np.float64) / HD))
    ang = np.outer(np.arange(S, dtype=np.float64), freqs)  # [S, 32]
    cosb = np.zeros((128, S), np.float32)
    sinb = np.zeros((128, S), np.float32)
    for row in range(128):
        q, j = divmod(row, 32)
        fi = (q % 2) * 16 + (j % 16)
        cosb[row] = np.cos(ang[:, fi])
        sinb[row] = (-1.0 if j < 16 else 1.0) * np.sin(ang[:, fi])
    kp = np.arange(128)[:, None]
    qf = np.arange(128)[None, :]
    masks = (kp <= qf).astype(np.float32)  # [128, 128] lower triangle
    sel8 = np.zeros((8, 512), np.float32)
    for i in range(8):
        sel8[i, i * 64:(i + 1) * 64] = 1.0
    return cosb, sinb, masks, sel8


def _build_program():
    import concourse.bass as bass
    import concourse.mybir as mybir
    import concourse.tile as tile
    from concourse import bacc
    from concourse.masks import make_identity
    from contextlib import ExitStack

    f32 = mybir.dt.float32
    f32r = mybir.dt.float32r
    bf16 = mybir.dt.bfloat16
    EXP = mybir.ActivationFunctionType.Exp
    MUL = mybir.AluOpType.mult
    ADD = mybir.AluOpType.add

    nc = bacc.Bacc("TRN2", target_bir_lowering=False, debug=False,
                   enable_asserts=False, num_devices=N_CORES)

    xt_d = nc.dram_tensor("xt", [DIM, S], bf16, kind="ExternalInput").ap()
    wq_d = nc.dram_tensor("wq", [DIM, 512], bf16, kind="ExternalInput").ap()
    wk_d = nc.dram_tensor("wk", [DIM, 128], bf16, kind="ExternalInput").ap()
    wv_d = nc.dram_tensor("wv", [DIM, 128], bf16, kind="ExternalInput").ap()
    wo_d = nc.dram_tensor("wo", [512, DIM], bf16, kind="ExternalInput").ap()
    cos_d = nc.dram_tensor("cosb", [128, S], f32, kind="ExternalInput").ap()
    sin_d = nc.dram_tensor("sinb", [128, S], f32, kind="ExternalInput").ap()
    msk_d = nc.dram_tensor("masks", [128, 128], bf16,
                           kind="ExternalInput").ap()
    sel8_d = nc.dram_tensor("sel8", [8, 512], f32r,
                            kind="ExternalInput").ap()
    out_d = nc.dram_tensor("out", [S, DIM], f32, kind="ExternalOutput").ap()

    with tile.TileContext(nc) as tc, ExitStack() as top:
        const = top.enter_context(tc.tile_pool(name="const", bufs=1))
        persist = top.enter_context(tc.tile_pool(name="persist", bufs=1))
        wpool = top.enter_context(tc.tile_pool(name="wpool", bufs=1))
        xpool = top.enter_context(tc.tile_pool(name="xpool", bufs=28))
        qpool = top.enter_context(tc.tile_pool(name="qpool", bufs=2))
        atpool = top.enter_context(tc.tile_pool(name="atpool", bufs=2))
        rpool = top.enter_context(tc.tile_pool(name="rpool", bufs=3))
        vtpool = top.enter_context(tc.tile_pool(name="vtpool", bufs=1))
        epool = top.enter_context(tc.tile_pool(name="epool", bufs=5))
        rcpool = top.enter_context(tc.tile_pool(name="rcpool", bufs=2))
        oepool = top.enter_context(tc.tile_pool(name="oepool", bufs=3))
        # one shared PSUM pool, 8 banks via tag aliasing:
        #   q0,q1: QKV accumulators (also V-transpose + WO po via aliasing)
        #   s: merged score tiles [128,1024] = 2 banks x 2 bufs
        #   oa,ob: attention accumulators (WO po aliases these)
        psum = top.enter_context(tc.tile_pool(name="psum", bufs=1,
                                              space="PSUM"))

        # ---- weights + x are on the critical path: emit their DMAs first
        wq_sb = wpool.tile([128, DT, 512], bf16, tag="wq")
        wk_sb = wpool.tile([128, DT, 128], bf16, tag="wk")
        wv_sb = wpool.tile([128, DT, 128], bf16, tag="wv")
        wq_r = wq_d.rearrange("(t p) c -> p t c", p=128)
        wk_r = wk_d.rearrange("(t p) c -> p t c", p=128)
        wv_r = wv_d.rearrange("(t p) c -> p t c", p=128)
        # compute starts with the K/V chains, so their weights and x go
        # out first; wq follows (not needed until the Q passes)
        for h in range(4):
            sl = slice(h * DT // 4, (h + 1) * DT // 4)
            nc.sync.dma_start(wk_sb[:, sl, :], wk_r[:, sl, :])
            nc.sync.dma_start(wv_sb[:, sl, :], wv_r[:, sl, :])
        xts0 = []
        for d in range(DT):
            xt = xpool.tile([128, SCH], bf16, tag="x", name=f"x_0_{d}")
            nc.sync.dma_start(xt[:], xt_d[d * 128:(d + 1) * 128, 0:SCH])
            xts0.append(xt)
        for d in range(DT):
            nc.sync.dma_start(wq_sb[:, d, :], wq_r[:, d, :])

        # ---- constants ----
        cos_sb = const.tile([128, S], f32, tag="cos")
        sin_sb = const.tile([128, S], f32, tag="sin")
        msk_sb = const.tile([128, 128], bf16, tag="msk")
        nc.sync.dma_start(msk_sb[:], msk_d)
        for h in range(2):
            sl = slice(h * S // 2, (h + 1) * S // 2)
            nc.sync.dma_start(cos_sb[:, sl], cos_d[:, sl])
            nc.sync.dma_start(sin_sb[:, sl], sin_d[:, sl])
        ident = const.tile([128, 128], f32, tag="ident")
        make_identity(nc, ident[:])
        onecol_f = const.tile([128, 1], f32, tag="onecol_f")
        nc.vector.memset(onecol_f[:], 1.0)
        # sel8[j, i*64+m] = 1 if j==i else 0: K=8 selector matmul broadcasts
        # row i of an [8,512] rhs to 64 output partitions (lhsT base must be
        # 0/32/64, so single-row lhsT tiles at partition i are not legal)
        sel8 = const.tile([8, 512], f32r, tag="sel8")
        nc.sync.dma_start(sel8[:], sel8_d)
        # wo fits in SBUF in bf16 (16KB/partition): load it once, on the
        # idle Pool DMA queue so it never delays the x/wq startup stream
        wo_sb = wpool.tile([128, 4, DIM], bf16, tag="wo")
        wo_r = wo_d.rearrange("(g p) c -> p g c", p=128)
        for g in range(4):
            for h in range(2):
                hs = slice(h * DIM // 2, (h + 1) * DIM // 2)
                nc.sync.dma_start(wo_sb[:, g, hs], wo_r[:, g, hs])

        # ---- persistent activations ----
        kt_sb = [persist.tile([128, SCH], bf16, tag=f"kt{c}", name=f"kt{c}")
                 for c in range(NSCH)]
        vp_sb = [persist.tile([128, 130], bf16, tag=f"vp{t}", name=f"vp{t}")
                 for t in range(ST)]
        for t in range(ST):
            nc.scalar.copy(vp_sb[t][:, 64:65], onecol_f[:])
            nc.scalar.copy(vp_sb[t][:, 129:130], onecol_f[:])

        def rope_evac(ps, dst, cosc, sinc, nm):
            # dst = ps*cos + shuffle(ps)*sin ; the SBUF-only sin-multiply
            # runs on the idle Pool engine, the rest on DVE.
            t1 = rpool.tile([128, SCH], f32, tag="r1", name=f"r1_{nm}")
            nc.vector.stream_shuffle(t1[:], ps[:], mask=SHUF_MASK)
            nc.vector.tensor_tensor(dst, ps[:], cosc, MUL)
            t2 = rpool.tile([128, SCH], bf16, tag="r2", name=f"r2_{nm}")
            nc.gpsimd.tensor_tensor(t2[:], t1[:], sinc, MUL)
            nc.vector.tensor_tensor(dst, dst, t2[:], ADD)

        for c in range(NSCH):
            cs = slice(c * SCH, (c + 1) * SCH)
            cosc, sinc = cos_sb[:, cs], sin_sb[:, cs]
            # ---- x tiles for this chunk (chunk 0 was prefetched) ----
            if c == 0:
                xts = xts0
            else:
                xts = []
                for d in range(DT):
                    xt = xpool.tile([128, SCH], bf16, tag="x",
                                    name=f"x_{c}_{d}")
                    nc.sync.dma_start(xt[:], xt_d[d * 128:(d + 1) * 128, cs])
                    xts.append(xt)
            # per-chunk query tiles (freed after this chunk's attention)
            qt_c = [qpool.tile([128, SCH], bf16, tag=f"qt{g}",
                               name=f"qt{g}_{c}") for g in range(4)]
            # K/V chains first so kt/vp are ready the moment attention
            # starts; then the Q passes (attention g0 needs only qt_c[0])
            psk = psum.tile([128, SCH], f32, tag="q0", name=f"psk_{c}")
            psv = psum.tile([128, SCH], f32, tag="q1", name=f"psv_{c}")
            for d in range(DT):
                st, sp = (d == 0), (d == DT - 1)
                nc.tensor.matmul(psk[:], wk_sb[:, d, :], xts[d][:],
                                 start=st, stop=sp)
                nc.tensor.matmul(psv[:], wv_sb[:, d, :], xts[d][:],
                                 start=st, stop=sp)
            rope_evac(psk, kt_sb[c][:], cosc, sinc, f"k{c}")
            vt = vtpool.tile([128, SCH], f32, tag="vt", name=f"vt_{c}")
            nc.scalar.copy(vt[:], psv[:])
            for rr in range(4):
                kt_i = 4 * c + rr
                pst = psum.tile([128, 128], f32, tag="q0",
                                name=f"pst_{c}_{rr}")
                nc.tensor.transpose(pst[:], vt[:, rr * 128:(rr + 1) * 128],
                                    ident[:])
                nc.scalar.copy(vp_sb[kt_i][:, 0:64], pst[:, 0:64])
                nc.scalar.copy(vp_sb[kt_i][:, 65:129], pst[:, 64:128])
            for g in range(2):
                ps0 = psum.tile([128, SCH], f32, tag="q0",
                                name=f"psq{2*g}_{c}")
                ps1 = psum.tile([128, SCH], f32, tag="q1",
                                name=f"psq{2*g+1}_{c}")
                for d in range(DT):
                    st, sp = (d == 0), (d == DT - 1)
                    nc.tensor.matmul(
                        ps0[:], wq_sb[:, d, 2 * g * 128:(2 * g + 1) * 128],
                        xts[d][:], start=st, stop=sp)
                    nc.tensor.matmul(
                        ps1[:],
                        wq_sb[:, d, (2 * g + 1) * 128:(2 * g + 2) * 128],
                        xts[d][:], start=st, stop=sp)
                rope_evac(ps0, qt_c[2 * g][:], cosc, sinc, f"a{c}_{2*g}")
                rope_evac(ps1, qt_c[2 * g + 1][:], cosc, sinc,
                          f"a{c}_{2*g+1}")

            # ---- attention for this chunk ----
            nkt = 4 * (c + 1)
            at_c = [atpool.tile([128, SCH], bf16, tag=f"at{g}",
                                name=f"at{g}_{c}") for g in range(4)]
            accs = []
            den8 = rcpool.tile([8, SCH], f32, tag="den8", name=f"den8_{c}")
            for g in range(4):
                pa = psum.tile([65, SCH], f32, tag="oa", name=f"oa_{c}_{g}")
                pb = psum.tile([65, SCH], f32, tag="ob", name=f"ob_{c}_{g}")

                def emit_scores(t):
                    rr = t - 4 * c
                    lo = max(rr, 0) * 128  # causally-live columns start here
                    qs = slice(lo, SCH)
                    kc, ko = t // 4, (t % 4) * 128
                    ktt = kt_sb[kc][:, ko:ko + 128]
                    s2 = psum.tile([128, 2 * SCH], f32, tag="s", bufs=2,
                                   name=f"s_{c}_{g}_{t}")
                    nc.tensor.matmul(s2[:, lo:SCH], ktt[0:64, :],
                                     qt_c[g][0:64, qs],
                                     start=True, stop=True)
                    nc.tensor.matmul(s2[:, SCH + lo:2 * SCH], ktt[64:128, :],
                                     qt_c[g][64:128, qs],
                                     start=True, stop=True)
                    e2 = epool.tile([128, 2 * SCH], bf16, tag="e",
                                    name=f"e_{c}_{g}_{t}")
                    # exp per head-half: AV(head0) depends only on the first
                    # exp, halving the exp->AV latency the PE queue eats;
                    # the head0 mask (DVE) overlaps the head1 exp (ACT)
                    nc.scalar.activation(e2[:, lo:SCH], s2[:, lo:SCH],
                                         EXP, scale=0.125)
                    if rr >= 0:
                        mb = slice(lo, lo + 128)
                        nc.vector.tensor_tensor(e2[:, mb], e2[:, mb],
                                                msk_sb[:], MUL)
                    nc.scalar.activation(e2[:, SCH + lo:2 * SCH],
                                         s2[:, SCH + lo:2 * SCH],
                                         EXP, scale=0.125)
                    if rr >= 0:
                        mb2 = slice(SCH + lo, SCH + lo + 128)
                        nc.vector.tensor_tensor(e2[:, mb2], e2[:, mb2],
                                                msk_sb[:], MUL)
                    return e2, lo

                def emit_av(t, e2, lo):
                    st, sp = (t == 0), (t == nkt - 1)
                    nc.tensor.matmul(pa[:, lo:], vp_sb[t][:, 0:65],
                                     e2[:, lo:SCH], start=st, stop=sp)
                    nc.tensor.matmul(pb[:, lo:], vp_sb[t][:, 65:130],
                                     e2[:, SCH + lo:2 * SCH],
                                     start=st, stop=sp)

                # software-pipelined: scores for t+1 are emitted before AV
                # of t so a dep-waiting AV never blocks the next scores at
                # the head of the in-order PE queue
                prev = emit_scores(0)
                for t in range(1, nkt):
                    cur = emit_scores(t)
                    emit_av(t - 1, *prev)
                    prev = cur
                emit_av(nkt - 1, *prev)
                # evacuate the accumulators; normalization happens batched
                # per chunk (one reciprocal over all 8 denominator rows)
                for half, ps in ((0, pa), (1, pb)):
                    i = 2 * g + half
                    acc = rcpool.tile([65, SCH], f32, tag="acc", bufs=8,
                                      name=f"acc{half}_{c}_{g}")
                    nc.vector.tensor_copy(acc[:], ps[:])
                    nc.sync.dma_start(den8[i:i + 1, :], acc[64:65, :])
                    accs.append(acc)

            # ---- batched softmax normalization for this chunk ----
            # gather the 8 denominator rows, one exact reciprocal, then
            # broadcast each row via a K=1 ones matmul and scale into at.
            rc8r = rcpool.tile([8, SCH], f32r, tag="rc8r", name=f"rc8r_{c}")
            with nc.allow_low_precision(reason="f32r is 32-bit; reciprocal "
                                        "output feeds a matmul rhs"):
                nc.vector.reciprocal(rc8r[:], den8[:])
            for g in range(4):
                bc = psum.tile([128, SCH], f32, tag="s", bufs=2,
                               name=f"bc_{c}_{g}")
                nc.tensor.matmul(bc[:], sel8[:, g * 128:(g + 1) * 128],
                                 rc8r[:], start=True, stop=True)
                for half in range(2):
                    acc = accs[2 * g + half]
                    dst = at_c[g][half * 64:(half + 1) * 64, :]
                    nc.vector.tensor_tensor(
                        dst, acc[0:64, :],
                        bc[half * 64:(half + 1) * 64, :], MUL)

            # ---- output projection for this chunk ----
            for e in range(4):
                for m in range(4):
                    ms = slice((4 * c + m) * 128, (4 * c + m + 1) * 128)
                    po = psum.tile([128, 512], f32,
                                   tag="oa" if (m + e) % 2 == 0 else "ob",
                                   name=f"po_{4*c+m}_{e}")
                    for g in range(4):
                        nc.tensor.matmul(
                            po[:], at_c[g][:, m * 128:(m + 1) * 128],
                            wo_sb[:, g, e * 512:(e + 1) * 512],
                            start=(g == 0), stop=(g == 3))
                    ot = oepool.tile([128, 512], f32, tag="ot",
                                     name=f"ot_{4*c+m}_{e}")
                    nc.vector.tensor_copy(ot[:], po[:])
                    nc.gpsimd.dma_start(out_d[ms, e * 512:(e + 1) * 512],
                                        ot[:])

    nc.compile()
    return nc


def get_program():
    if "nc" not in _CACHE:
        _CACHE["nc"] = _build_program()
    return _CACHE["nc"]


def shard_inputs(x, wq, wk, wv, wo):
    """Returns in_maps for cores 0..7; core = b*4 + g."""
    cosb, sinb, masks, sel8 = _host_constants()
    in_maps = []
    for b in range(B):
        xT = np.ascontiguousarray(
            np.asarray(x[b], np.float32).T.astype(ml_dtypes.bfloat16))
        for g in range(TPG):
            qheads = [H_CORE * g + h for h in HEAD_ORDER_LOCAL]
            qcols = np.concatenate([h * HD + PERM64 for h in qheads])
            kvheads = [KV_CORE * g, KV_CORE * g + 1]
            kcols = np.concatenate([h * HD + PERM64 for h in kvheads])
            vcols = np.concatenate([h * HD + np.arange(HD) for h in kvheads])
            worows = np.concatenate([h * HD + np.arange(HD) for h in qheads])
            in_maps.append({
                "xt": xT,
                "wq": np.ascontiguousarray(
                    np.asarray(wq, np.float32)[:, qcols].astype(
                        ml_dtypes.bfloat16)),
                "wk": np.ascontiguousarray(
                    np.asarray(wk, np.float32)[:, kcols].astype(
                        ml_dtypes.bfloat16)),
                "wv": np.ascontiguousarray(
                    np.asarray(wv, np.float32)[:, vcols].astype(
                        ml_dtypes.bfloat16)),
                "wo": np.ascontiguousarray(
                    np.asarray(wo, np.float32)[worows, :].astype(
                        ml_dtypes.bfloat16)),
                "cosb": cosb,
                "sinb": sinb,
                "masks": masks.astype(ml_dtypes.bfloat16),
                "sel8": sel8,
            })
    return in_maps


def _install_trace_shim():
    """Synthesize the antenv.axon_hooks NTFF profile hook (this image's
    antenv lacks it) so trace=True works under axon. Safe no-op on any
    failure."""
    import sys
    import types
    try:
        import antenv
        if getattr(antenv, "axon_hooks", None) is not None:
            return
        from trn_agent_boot.trn_boot import _ntff_profile_via_ctypes
        hook = _ntff_profile_via_ctypes("/opt/axon/libaxon_pjrt.so")
        mod = types.ModuleType("antenv.axon_hooks")
        mod.get_axon_ntff_profile_hook = lambda: hook
        mod.set_axon_ntff_profile_hook = lambda h: None
        sys.modules["antenv.axon_hooks"] = mod
        antenv.axon_hooks = mod
        from concourse import bass_utils
        bass_utils.upload_artifacts = lambda tmpdir: "local://unuploaded"
    except Exception as e:  # pragma: no cover
        print(f"trace shim unavailable: {e}")


def kernel(x, wq, wk, wv, wo):
    from concourse import bass_utils

    nc = get_program()
    in_maps = shard_inputs(x, wq, wk, wv, wo)
    trace = os.environ.get("KERNEL_TRACE", "0") == "1"
    if trace:
        _install_trace_shim()
    res = bass_utils.run_bass_kernel_spmd(
        nc, in_maps, core_ids=list(range(N_CORES)), trace=trace)
    LAST_RUN_INFO.clear()
    LAST_RUN_INFO.update(
        exec_time_ns=res.exec_time_ns,
        mean_exec_time_ns=res.mean_exec_time_ns,
        trace=(res.instructions_and_trace[1]
               if res.instructions_and_trace else None),
    )
    out = np.zeros((B, S, DIM), np.float32)
    for b in range(B):
        for g in range(TPG):
            out[b] += res.results[b * TPG + g]["out"]
    return out


def time_device_exec(inputs, iters=6):
    """Test-only: time warm PJRT executes with device-resident inputs.
    Returns per-iteration wall seconds (upper bound on device exec)."""
    import jax
    import concourse.mybir as mybir
    from jax.sharding import Mesh, PartitionSpec
    from jax.experimental.shard_map import shard_map
    from concourse.bass2jax import (_bass_exec_p, partition_id_tensor,
                                    install_neuronx_cc_hook)
    import time as _time

    install_neuronx_cc_hook()
    nc = get_program()
    in_maps = shard_inputs(**inputs) if isinstance(inputs, dict) else inputs

    partition_name = (nc.partition_id_tensor.name
                      if nc.partition_id_tensor else None)
    in_names, out_names, out_avals, zero_outs = [], [], [], []
    for alloc in nc.m.functions[0].allocations:
        if not isinstance(alloc, mybir.MemoryLocationSet):
            continue
        name = alloc.memorylocations[0].name
        if alloc.kind == "ExternalInput":
            if name != partition_name:
                in_names.append(name)
        elif alloc.kind == "ExternalOutput":
            shape = tuple(alloc.tensor_shape)
            dtype = mybir.dt.np(alloc.dtype)
            out_names.append(name)
            out_avals.append(jax.core.ShapedArray(shape, dtype))
            zero_outs.append(np.zeros(shape, dtype))
    n_params = len(in_names)
    n_outs = len(out_avals)
    all_in_names = list(in_names) + list(out_names)
    if partition_name is not None:
        all_in_names.append(partition_name)
    donate = tuple(range(n_params, n_params + n_outs))

    def _body(*args):
        operands = list(args)
        if partition_name is not None:
            operands.append(partition_id_tensor())
        outs = _bass_exec_p.bind(
            *operands, out_avals=tuple(out_avals),
            in_names=tuple(all_in_names), out_names=tuple(out_names),
            lowering_input_output_aliases=(), sim_require_finite=True,
            sim_require_nnan=True, nc=nc)
        return tuple(outs)

    devices = jax.devices()[:N_CORES]
    mesh = Mesh(np.asarray(devices), ("core",))
    sharded = jax.jit(
        shard_map(_body, mesh=mesh,
                  in_specs=(PartitionSpec("core"),) * (n_params + n_outs),
                  out_specs=(PartitionSpec("core"),) * n_outs,
                  check_rep=False),
        donate_argnums=donate, keep_unused=True)

    sh = jax.sharding.NamedSharding(mesh, PartitionSpec("core"))
    concat_in = [np.concatenate([np.asarray(in_maps[c][nm])
                                 for c in range(N_CORES)], axis=0)
                 for nm in in_names]
    in_dev = [jax.device_put(a, sh) for a in concat_in]
    for a in in_dev:
        a.block_until_ready()
    times = []
    for _ in range(iters):
        zs = [jax.device_put(np.zeros((N_CORES * z.shape[0], *z.shape[1:]),
                                      z.dtype), sh) for z in zero_outs]
        for z in zs:
            z.block_until_ready()
        t0 = _time.time()
        outs = sharded(*in_dev, *zs)
        for o in outs:
            o.block_until_ready()
        times.append(_time.time() - t0)
    return times


# revision 62
# speedup vs baseline: 1.2480x; 1.2480x over previous
# Trainium2 Bass kernel for GQA causal attention (B=2, S=2048, DIM=2048,
# NH=32, NKV=8, HD=64) sharded over 8 NeuronCores: 2-way data parallel over
# batch x 4-way tensor parallel over heads. Each core computes 8 query heads
# (2 KV heads) for one batch element plus a partial wo product; the partial
# sums are reduced on the host (cheap fp32 adds), so no device collective is
# needed.
#
# Structure: per chunk, K/V projection chains run first (so kt/vp are ready
# the moment attention starts), then Q chains + RoPE into per-chunk query
# tiles (no false cross-chunk deps). Attention uses merged [128,1024] score
# PSUM tiles (both heads of a GQA pair) with a single EXP per key tile and a
# software-pipelined t-loop (scores for t+1 emitted before AV of t so a
# dep-waiting AV never blocks the in-order PE queue). Softmax normalization
# is batched per chunk: one [8,512] reciprocal over all denominator rows
# (gathered by SBUF->SBUF DMAs; compute engines cannot write partitions at
# base 1..7), broadcast via K=8 selector matmuls. The whole data path is
# bf16 (fp32 PSUM accumulation), which enables FWL weight loads and 2x DVE
# modes; wo stays resident in SBUF. Measured ~404us on-device (NTFF).
#
# Self-contained: hardcodes all shapes; only imports the concourse runtime
# available in the environment.
import os
import ml_dtypes
import numpy as np

B, S, DIM = 2, 2048, 2048
NH, NKV, HD = 32, 8, 64
THETA = 10000.0
TPG = 4               # tensor-parallel head-group shards
H_CORE = NH // TPG    # 8 query heads per core
KV_CORE = NKV // TPG  # 2 kv heads per core
SCH = 512             # sequence chunk (matmul moving free dim)
NSCH = S // SCH       # 4
DT = DIM // 128       # 16 contraction tiles for projections
ST = S // 128         # 16 key tiles
N_CORES = 8

# within-head dim permutation: [e0(16) o0(16) e1(16) o1(16)] so that the RoPE
# partner lives 16 partitions away inside each 32-partition quadrant
# (stream_shuffle shuffles within 32-partition quadrants only).
PERM64 = np.array([2 * i for i in range(16)] + [2 * i + 1 for i in range(16)]
                  + [32 + 2 * i for i in range(16)]
                  + [33 + 2 * i for i in range(16)])
HEAD_ORDER_LOCAL = [0, 4, 1, 5, 2, 6, 3, 7]  # (p, p+4) share a 128-row tile
SHUF_MASK = [i ^ 16 for i in range(32)]

_CACHE: dict = {}
LAST_RUN_INFO: dict = {}


def _host_constants():
    freqs = 1.0 / (THETA ** (np.arange(0, HD, 2, dtype=np.float64) / HD))
    ang = np.outer(np.arange(S, dtype=np.float64), freqs)  # [S, 32]
    cosb = np.zeros((128, S), np.float32)
    sinb = np.zeros((128, S), np.float32)
    for row in range(128):
        q, j = divmod(row, 32)
        fi = (q % 2) * 16 + (j % 16)
        cosb[row] = np.cos(ang[:, fi])
        sinb[row] = (-1.0 if j < 16 else 1.0) * np.sin(ang[:, fi])
    kp = np.arange(128)[:, None]
    qf = np.arange(128)[None, :]
    masks = (kp <= qf).astype(np.float32)  # [128, 128] lower triangle
    sel8 = np.zeros((8, 512), np.float32)
    for i in range(8):
        sel8[i, i * 64:(i + 1) * 64] = 1.0
    return cosb, sinb, masks, sel8


def _build_program():
    import concourse.bass as bass
    import concourse.mybir as mybir
    import concourse.tile as tile
    from concourse import bacc
    from concourse.masks import make_identity
    from contextlib import ExitStack

    f32 = mybir.dt.float32
    f32r = mybir.dt.float32r
    bf16 = mybir.dt.bfloat16
    EXP = mybir.ActivationFunctionType.Exp
    MUL = mybir.AluOpType.mult
    ADD = mybir.AluOpType.add

    nc = bacc.Bacc("TRN2", target_bir_lowering=False, debug=False,
                   enable_asserts=False, num_devices=N_CORES)

    xt_d = nc.dram_tensor("xt", [DIM, S], bf16, kind="ExternalInput").ap()
    wq_d = nc.dram_tensor("wq", [DIM, 512], bf16, kind="ExternalInput").ap()
    wk_d = nc.dram_tensor("wk", [DIM, 128], bf16, kind="ExternalInput").ap()
    wv_d = nc.dram_tensor("wv", [DIM, 128], bf16, kind="ExternalInput").ap()
    wo_d = nc.dram_tensor("wo", [512, DIM], bf16, kind="ExternalInput").ap()
    cos_d = nc.dram_tensor("cosb", [128, S], f32, kind="ExternalInput").ap()
    sin_d = nc.dram_tensor("sinb", [128, S], f32, kind="ExternalInput").ap()
    msk_d = nc.dram_tensor("masks", [128, 128], bf16,
                           kind="ExternalInput").ap()
    sel8_d = nc.dram_tensor("sel8", [8, 512], f32r,
                            kind="ExternalInput").ap()
    out_d = nc.dram_tensor("out", [S, DIM], f32, kind="ExternalOutput").ap()

    with tile.TileContext(nc) as tc, ExitStack() as top:
        const = top.enter_context(tc.tile_pool(name="const", bufs=1))
        persist = top.enter_context(tc.tile_pool(name="persist", bufs=1))
        wpool = top.enter_context(tc.tile_pool(name="wpool", bufs=1))
        xpool = top.enter_context(tc.tile_pool(name="xpool", bufs=28))
        qpool = top.enter_context(tc.tile_pool(name="qpool", bufs=2))
        atpool = top.enter_context(tc.tile_pool(name="atpool", bufs=2))
        rpool = top.enter_context(tc.tile_pool(name="rpool", bufs=3))
        vtpool = top.enter_context(tc.tile_pool(name="vtpool", bufs=1))
        epool = top.enter_context(tc.tile_pool(name="epool", bufs=5))
        rcpool = top.enter_context(tc.tile_pool(name="rcpool", bufs=2))
        oepool = top.enter_context(tc.tile_pool(name="oepool", bufs=3))
        # one shared PSUM pool, 8 banks via tag aliasing:
        #   q0,q1: QKV accumulators (also V-transpose + WO po via aliasing)
        #   s: merged score tiles [128,1024] = 2 banks x 2 bufs
        #   oa,ob: attention accumulators (WO po aliases these)
        psum = top.enter_context(tc.tile_pool(name="psum", bufs=1,
                                              space="PSUM"))

        # ---- weights + x are on the critical path: emit their DMAs first
        wq_sb = wpool.tile([128, DT, 512], bf16, tag="wq")
        wk_sb = wpool.tile([128, DT, 128], bf16, tag="wk")
        wv_sb = wpool.tile([128, DT, 128], bf16, tag="wv")
        wq_r = wq_d.rearrange("(t p) c -> p t c", p=128)
        wk_r = wk_d.rearrange("(t p) c -> p t c", p=128)
        wv_r = wv_d.rearrange("(t p) c -> p t c", p=128)
        # compute starts with the K/V chains, so their weights and x go
        # out first; wq follows (not needed until the Q passes)
        for h in range(4):
            sl = slice(h * DT // 4, (h + 1) * DT // 4)
            nc.sync.dma_start(wk_sb[:, sl, :], wk_r[:, sl, :])
            nc.sync.dma_start(wv_sb[:, sl, :], wv_r[:, sl, :])
        xts0 = []
        for d in range(DT):
            xt = xpool.tile([128, SCH], bf16, tag="x", name=f"x_0_{d}")
            nc.sync.dma_start(xt[:], xt_d[d * 128:(d + 1) * 128, 0:SCH])
            xts0.append(xt)
        for d in range(DT):
            nc.sync.dma_start(wq_sb[:, d, :], wq_r[:, d, :])

        # ---- constants ----
        cos_sb = const.tile([128, S], f32, tag="cos")
        sin_sb = const.tile([128, S], f32, tag="sin")
        msk_sb = const.tile([128, 128], bf16, tag="msk")
        nc.sync.dma_start(msk_sb[:], msk_d)
        for h in range(2):
            sl = slice(h * S // 2, (h + 1) * S // 2)
            nc.sync.dma_start(cos_sb[:, sl], cos_d[:, sl])
            nc.sync.dma_start(sin_sb[:, sl], sin_d[:, sl])
        ident = const.tile([128, 128], f32, tag="ident")
        make_identity(nc, ident[:])
        onecol_f = const.tile([128, 1], f32, tag="onecol_f")
        nc.vector.memset(onecol_f[:], 1.0)
        # sel8[j, i*64+m] = 1 if j==i else 0: K=8 selector matmul broadcasts
        # row i of an [8,512] rhs to 64 output partitions (lhsT base must be
        # 0/32/64, so single-row lhsT tiles at partition i are not legal)
        sel8 = const.tile([8, 512], f32r, tag="sel8")
        nc.sync.dma_start(sel8[:], sel8_d)
        # wo fits in SBUF in bf16 (16KB/partition): load it once, on the
        # idle Pool DMA queue so it never delays the x/wq startup stream
        wo_sb = wpool.tile([128, 4, DIM], bf16, tag="wo")
        wo_r = wo_d.rearrange("(g p) c -> p g c", p=128)
        for g in range(4):
            for h in range(2):
                hs = slice(h * DIM // 2, (h + 1) * DIM // 2)
                nc.sync.dma_start(wo_sb[:, g, hs], wo_r[:, g, hs])

        # ---- persistent activations ----
        kt_sb = [persist.tile([128, SCH], bf16, tag=f"kt{c}", name=f"kt{c}")
                 for c in range(NSCH)]
        vp_sb = [persist.tile([128, 130], bf16, tag=f"vp{t}", name=f"vp{t}")
                 for t in range(ST)]
        for t in range(ST):
            nc.scalar.copy(vp_sb[t][:, 64:65], onecol_f[:])
            nc.scalar.copy(vp_sb[t][:, 129:130], onecol_f[:])

        def rope_evac(ps, dst, cosc, sinc, nm):
            # dst = ps*cos + shuffle(ps)*sin ; the SBUF-only sin-multiply
            # runs on the idle Pool engine, the rest on DVE.
            t1 = rpool.tile([128, SCH], f32, tag="r1", name=f"r1_{nm}")
            nc.vector.stream_shuffle(t1[:], ps[:], mask=SHUF_MASK)
            nc.vector.tensor_tensor(dst, ps[:], cosc, MUL)
            t2 = rpool.tile([128, SCH], bf16, tag="r2", name=f"r2_{nm}")
            nc.gpsimd.tensor_tensor(t2[:], t1[:], sinc, MUL)
            nc.vector.tensor_tensor(dst, dst, t2[:], ADD)

        for c in range(NSCH):
            cs = slice(c * SCH, (c + 1) * SCH)
            cosc, sinc = cos_sb[:, cs], sin_sb[:, cs]
            # ---- x tiles for this chunk (chunk 0 was prefetched) ----
            if c == 0:
                xts = xts0
            else:
                xts = []
                for d in range(DT):
                    xt = xpool.tile([128, SCH], bf16, tag="x",
                                    name=f"x_{c}_{d}")
                    nc.sync.dma_start(xt[:], xt_d[d * 128:(d + 1) * 128, cs])
                    xts.append(xt)
            # per-chunk query tiles (freed after this chunk's attention)
            qt_c = [qpool.tile([128, SCH], bf16, tag=f"qt{g}",
                               name=f"qt{g}_{c}") for g in range(4)]
            # K/V chains first so kt/vp are ready the moment attention
            # starts; then the Q passes (attention g0 needs only qt_c[0])
            psk = psum.tile([128, SCH], f32, tag="q0", name=f"psk_{c}")
            psv = psum.tile([128, SCH], f32, tag="q1", name=f"psv_{c}")
            for d in range(DT):
                st, sp = (d == 0), (d == DT - 1)
                nc.tensor.matmul(psk[:], wk_sb[:, d, :], xts[d][:],
                                 start=st, stop=sp)
                nc.tensor.matmul(psv[:], wv_sb[:, d, :], xts[d][:],
                                 start=st, stop=sp)
            rope_evac(psk, kt_sb[c][:], cosc, sinc, f"k{c}")
            vt = vtpool.tile([128, SCH], f32, tag="vt", name=f"vt_{c}")
            nc.scalar.copy(vt[:], psv[:])
            for rr in range(4):
                kt_i = 4 * c + rr
                pst = psum.tile([128, 128], f32, tag="q0",
                                name=f"pst_{c}_{rr}")
                nc.tensor.transpose(pst[:], vt[:, rr * 128:(rr + 1) * 128],
                                    ident[:])
                nc.scalar.copy(vp_sb[kt_i][:, 0:64], pst[:, 0:64])
                nc.scalar.copy(vp_sb[kt_i][:, 65:129], pst[:, 64:128])
            for g in range(2):
                ps0 = psum.tile([128, SCH], f32, tag="q0",
                                name=f"psq{2*g}_{c}")
                ps1 = psum.tile([128, SCH], f32, tag="q1",
                                name=f"psq{2*g+1}_{c}")
                for d in range(DT):
                    st, sp = (d == 0), (d == DT - 1)
                    nc.tensor.matmul(
                        ps0[:], wq_sb[:, d, 2 * g * 128:(2 * g + 1) * 128],
                        xts[d][:], start=st, stop=sp)
                    nc.tensor.matmul(
                        ps1[:],
                        wq_sb[:, d, (2 * g + 1) * 128:(2 * g + 2) * 128],
                        xts[d][:], start=st, stop=sp)
                rope_evac(ps0, qt_c[2 * g][:], cosc, sinc, f"a{c}_{2*g}")
                rope_evac(ps1, qt_c[2 * g + 1][:], cosc, sinc,
                          f"a{c}_{2*g+1}")

            # ---- attention for this chunk ----
            nkt = 4 * (c + 1)
            at_c = [atpool.tile([128, SCH], bf16, tag=f"at{g}",
                                name=f"at{g}_{c}") for g in range(4)]
            accs = []
            den8 = rcpool.tile([8, SCH], f32, tag="den8", name=f"den8_{c}")
            for g in range(4):
                pa = psum.tile([65, SCH], f32, tag="oa", name=f"oa_{c}_{g}")
                pb = psum.tile([65, SCH], f32, tag="ob", name=f"ob_{c}_{g}")

                def emit_scores(t):
                    rr = t - 4 * c
                    lo = max(rr, 0) * 128  # causally-live columns start here
                    qs = slice(lo, SCH)
                    kc, ko = t // 4, (t % 4) * 128
                    ktt = kt_sb[kc][:, ko:ko + 128]
                    s2 = psum.tile([128, 2 * SCH], f32, tag="s", bufs=2,
                                   name=f"s_{c}_{g}_{t}")
                    nc.tensor.matmul(s2[:, lo:SCH], ktt[0:64, :],
                                     qt_c[g][0:64, qs],
                                     start=True, stop=True)
                    nc.tensor.matmul(s2[:, SCH + lo:2 * SCH], ktt[64:128, :],
                                     qt_c[g][64:128, qs],
                                     start=True, stop=True)
                    e2 = epool.tile([128, 2 * SCH], bf16, tag="e",
                                    name=f"e_{c}_{g}_{t}")
                    if lo == 0:
                        nc.scalar.activation(e2[:], s2[:], EXP, scale=0.125)
                    else:
                        sv = s2[:].rearrange("p (h n) -> p h n", h=2)[:, :,
                                                                     lo:]
                        ev = e2[:].rearrange("p (h n) -> p h n", h=2)[:, :,
                                                                     lo:]
                        nc.scalar.activation(ev, sv, EXP, scale=0.125)
                    if rr >= 0:  # mask the mixed 128-column block
                        mb = slice(lo, lo + 128)
                        mb2 = slice(SCH + lo, SCH + lo + 128)
                        nc.vector.tensor_tensor(e2[:, mb], e2[:, mb],
                                                msk_sb[:], MUL)
                        nc.vector.tensor_tensor(e2[:, mb2], e2[:, mb2],
                                                msk_sb[:], MUL)
                    return e2, lo

                def emit_av(t, e2, lo):
                    st, sp = (t == 0), (t == nkt - 1)
                    nc.tensor.matmul(pa[:, lo:], vp_sb[t][:, 0:65],
                                     e2[:, lo:SCH], start=st, stop=sp)
                    nc.tensor.matmul(pb[:, lo:], vp_sb[t][:, 65:130],
                                     e2[:, SCH + lo:2 * SCH],
                                     start=st, stop=sp)

                # software-pipelined: scores for t+1 are emitted before AV
                # of t so a dep-waiting AV never blocks the next scores at
                # the head of the in-order PE queue
                prev = emit_scores(0)
                for t in range(1, nkt):
                    cur = emit_scores(t)
                    emit_av(t - 1, *prev)
                    prev = cur
                emit_av(nkt - 1, *prev)
                # evacuate the accumulators; normalization happens batched
                # per chunk (one reciprocal over all 8 denominator rows)
                for half, ps in ((0, pa), (1, pb)):
                    i = 2 * g + half
                    acc = rcpool.tile([65, SCH], f32, tag="acc", bufs=8,
                                      name=f"acc{half}_{c}_{g}")
                    nc.vector.tensor_copy(acc[:], ps[:])
                    nc.sync.dma_start(den8[i:i + 1, :], acc[64:65, :])
                    accs.append(acc)

            # ---- batched softmax normalization for this chunk ----
            # gather the 8 denominator rows, one exact reciprocal, then
            # broadcast each row via a K=1 ones matmul and scale into at.
            rc8r = rcpool.tile([8, SCH], f32r, tag="rc8r", name=f"rc8r_{c}")
            with nc.allow_low_precision(reason="f32r is 32-bit; reciprocal "
                                        "output feeds a matmul rhs"):
                nc.vector.reciprocal(rc8r[:], den8[:])
            for g in range(4):
                bc = psum.tile([128, SCH], f32, tag="s", bufs=2,
                               name=f"bc_{c}_{g}")
                nc.tensor.matmul(bc[:], sel8[:, g * 128:(g + 1) * 128],
                                 rc8r[:], start=True, stop=True)
                for half in range(2):
                    acc = accs[2 * g + half]
                    dst = at_c[g][half * 64:(half + 1) * 64, :]
                    nc.vector.tensor_tensor(
                        dst, acc[0:64, :],
                        bc[half * 64:(half + 1) * 64, :], MUL)

            # ---- output projection for this chunk ----
            for e in range(4):
                for m in range(4):
                    ms = slice((4 * c + m) * 128, (4 * c + m + 1) * 128)
                    po = psum.tile([128, 512], f32,
                                   tag="oa" if (m + e) % 2 == 0 else "ob",
                                   name=f"po_{4*c+m}_{e}")
                    for g in range(4):
                        nc.tensor.matmul(
                            po[:], at_c[g][:, m * 128:(m + 1) * 128],
                            wo_sb[:, g, e * 512:(e + 1) * 512],
                            start=(g == 0), stop=(g == 3))
                    ot = oepool.tile([128, 512], f32, tag="ot",
                                     name=f"ot_{4*c+m}_{e}")
                    nc.vector.tensor_copy(ot[:], po[:])
                    nc.gpsimd.dma_start(out_d[ms, e * 512:(e + 1) * 512],
                                        ot[:])

    nc.compile()
    return nc


def get_program():
    if "nc" not in _CACHE:
        _CACHE["nc"] = _build_program()
    return _CACHE["nc"]


def shard_inputs(x, wq, wk, wv, wo):
    """Returns in_maps for cores 0..7; core = b*4 + g."""
    cosb, sinb, masks, sel8 = _host_constants()
    in_maps = []
    for b in range(B):
        xT = np.ascontiguousarray(
            np.asarray(x[b], np.float32).T.astype(ml_dtypes.bfloat16))
        for g in range(TPG):
            qheads = [H_CORE * g + h for h in HEAD_ORDER_LOCAL]
            qcols = np.concatenate([h * HD + PERM64 for h in qheads])
            kvheads = [KV_CORE * g, KV_CORE * g + 1]
            kcols = np.concatenate([h * HD + PERM64 for h in kvheads])
            vcols = np.concatenate([h * HD + np.arange(HD) for h in kvheads])
            worows = np.concatenate([h * HD + np.arange(HD) for h in qheads])
            in_maps.append({
                "xt": xT,
                "wq": np.ascontiguousarray(
                    np.asarray(wq, np.float32)[:, qcols].astype(
                        ml_dtypes.bfloat16)),
                "wk": np.ascontiguousarray(
                    np.asarray(wk, np.float32)[:, kcols].astype(
                        ml_dtypes.bfloat16)),
                "wv": np.ascontiguousarray(
                    np.asarray(wv, np.float32)[:, vcols].astype(
                        ml_dtypes.bfloat16)),
                "wo": np.ascontiguousarray(
                    np.asarray(wo, np.float32)[worows, :].astype(
                        ml_dtypes.bfloat16)),
                "cosb": cosb,
                "sinb": sinb,
                "masks": masks.astype(ml_dtypes.bfloat16),
                "sel8": sel8,
            })
    return in_maps


def _install_trace_shim():
    """Synthesize the antenv.axon_hooks NTFF profile hook (this image's
    antenv lacks it) so trace=True works under axon. Safe no-op on any
    failure."""
    import sys
    import types
    try:
        import antenv
        if getattr(antenv, "axon_hooks", None) is not None:
            return
        from trn_agent_boot.trn_boot import _ntff_profile_via_ctypes
        hook = _ntff_profile_via_ctypes("/opt/axon/libaxon_pjrt.so")
        mod = types.ModuleType("antenv.axon_hooks")
        mod.get_axon_ntff_profile_hook = lambda: hook
        mod.set_axon_ntff_profile_hook = lambda h: None
        sys.modules["antenv.axon_hooks"] = mod
        antenv.axon_hooks = mod
        from concourse import bass_utils
        bass_utils.upload_artifacts = lambda tmpdir: "local://unuploaded"
    except Exception as e:  # pragma: no cover
        print(f"trace shim unavailable: {e}")


def kernel(x, wq, wk, wv, wo):
    from concourse import bass_utils

    nc = get_program()
    in_maps = shard_inputs(x, wq, wk, wv, wo)
    trace = os.environ.get("KERNEL_TRACE", "0") == "1"
    if trace:
        _install_trace_shim()
    res = bass_utils.run_bass_kernel_spmd(
        nc, in_maps, core_ids=list(range(N_CORES)), trace=trace)
    LAST_RUN_INFO.clear()
    LAST_RUN_INFO.update(
        exec_time_ns=res.exec_time_ns,
        mean_exec_time_ns=res.mean_exec_time_ns,
        trace=(res.instructions_and_trace[1]
               if res.instructions_and_trace else None),
    )
    out = np.zeros((B, S, DIM), np.float32)
    for b in range(B):
        for g in range(TPG):
            out[b] += res.results[b * TPG + g]["out"]
    return out


def time_device_exec(inputs, iters=6):
    """Test-only: time warm PJRT executes with device-resident inputs.
    Returns per-iteration wall seconds (upper bound on device exec)."""
    import jax
    import concourse.mybir as mybir
    from jax.sharding import Mesh, PartitionSpec
    from jax.experimental.shard_map import shard_map
    from concourse.bass2jax import (_bass_exec_p, partition_id_tensor,
                                    install_neuronx_cc_hook)
    import time as _time

    install_neuronx_cc_hook()
    nc = get_program()
    in_maps = shard_inputs(**inputs) if isinstance(inputs, dict) else inputs

    partition_name = (nc.partition_id_tensor.name
                      if nc.partition_id_tensor else None)
    in_names, out_names, out_avals, zero_outs = [], [], [], []
    for alloc in nc.m.functions[0].allocations:
        if not isinstance(alloc, mybir.MemoryLocationSet):
            continue
        name = alloc.memorylocations[0].name
        if alloc.kind == "ExternalInput":
            if name != partition_name:
                in_names.append(name)
        elif alloc.kind == "ExternalOutput":
            shape = tuple(alloc.tensor_shape)
            dtype = mybir.dt.np(alloc.dtype)
            out_names.append(name)
            out_avals.append(jax.core.ShapedArray(shape, dtype))
            zero_outs.append(np.zeros(shape, dtype))
    n_params = len(in_names)
    n_outs = len(out_avals)
    all_in_names = list(in_names) + list(out_names)
    if partition_name is not None:
        all_in_names.append(partition_name)
    donate = tuple(range(n_params, n_params + n_outs))

    def _body(*args):
        operands = list(args)
        if partition_name is not None:
            operands.append(partition_id_tensor())
        outs = _bass_exec_p.bind(
            *operands, out_avals=tuple(out_avals),
            in_names=tuple(all_in_names), out_names=tuple(out_names),
            lowering_input_output_aliases=(), sim_require_finite=True,
            sim_require_nnan=True, nc=nc)
        return tuple(outs)

    devices = jax.devices()[:N_CORES]
    mesh = Mesh(np.asarray(devices), ("core",))
    sharded = jax.jit(
        shard_map(_body, mesh=mesh,
                  in_specs=(PartitionSpec("core"),) * (n_params + n_outs),
                  out_specs=(PartitionSpec("core"),) * n_outs,
                  check_rep=False),
        donate_argnums=donate, keep_unused=True)

    sh = jax.sharding.NamedSharding(mesh, PartitionSpec("core"))
    concat_in = [np.concatenate([np.asarray(in_maps[c][nm])
                                 for c in range(N_CORES)], axis=0)
                 for nm in in_names]
    in_dev = [jax.device_put(a, sh) for a in concat_in]
    for a in in_dev:
        a.block_until_ready()
    times = []
    for _ in range(iters):
        zs = [jax.device_put(np.zeros((N_CORES * z.shape[0], *z.shape[1:]),
                                      z.dtype), sh) for z in zero_outs]
        for z in zs:
            z.block_until_ready()
        t0 = _time.time()
        outs = sharded(*in_dev, *zs)
        for o in outs:
            o.block_until_ready()
        times.append(_time.time() - t0)
    return times
